# revision 10
# baseline (speedup 1.0000x reference)
# Binarized CNN (MCNET) on 8 TRN2 NeuronCores — pure batch data-parallel.
#
# Math: reference net is
#   h = pad(x, 1, value=1)
#   c1 = conv3x3(h, sign(w1)); a2 = maxpool2(hardtanh(c1)); (sign taken later)
#   c2 = conv3x3(sign(a2), sign(w2)); ...
#   out = hardtanh(c4)
# hardtanh is monotone and sign(hardtanh(v)) == sign(v), and
# sign(maxpool(v)) == maxpool(sign(v)), so the net collapses to:
#   c1 (fp32) -> a2 = maxpool2(sign(c1)) in {-1,0,1}
#   c2 = conv(a2, sign(w2)) (exact small-integer arithmetic) -> a3 = sign(c2)
#   c3 = conv(a3, sign(w3)) -> a4 = sign(c3)
#   out = clip(conv(a4, sign(w4)), -1, 1)
# Layers 2-4 are exact in any fp format with fp32 accumulation (values are
# ternary, sums <= 288), so they run in bf16/fp8. Layer 1 runs in fp32.
#
# Each conv layer is a matmul with partitions K = (channel, replicated-row),
# free dim = (row-block, x). dy is baked into the K replicas ("row im2col"),
# dx becomes 3 PSUM-accumulated matmuls with shifted rhs. Layer 1 packs the
# 2x2 maxpool parity into the PSUM partition order so pooling is a
# contiguous-partition-range max plus a strided free-dim max.

import numpy as np
import ml_dtypes
from contextlib import ExitStack

import concourse.bass as bass
import concourse.mybir as mybir
import concourse.tile as tile
from concourse import bacc

F32 = mybir.dt.float32
BF16 = mybir.dt.bfloat16
FP8 = mybir.dt.float8e4

NP_BF16 = ml_dtypes.bfloat16
NP_FP8 = ml_dtypes.float8_e4m3


class Cfg:
    def __init__(self, B, H, W):
        assert H % 16 == 0 and W % 16 == 0
        self.B, self.H, self.W = B, H, W
        self.Hp, self.Wp = H + 2, W + 2
        # L1: 16 conv rows per block -> 8 pooled rows per block
        self.NB1 = H // 16
        self.H2, self.W2 = H // 2, W // 2          # a2 spatial
        self.N1 = W                                 # L1 matmul free dim
        # L2: Sy=8
        self.H3, self.W3 = self.H2 - 2, self.W2 - 2  # c2/a3 spatial
        self.NB2 = (self.H3 + 7) // 8
        self.N2 = self.W3
        # L3: Sy=4
        self.H4, self.W4 = self.H3 - 2, self.W3 - 2  # c3/a4 spatial
        assert self.H4 % 4 == 0
        self.NB3 = self.H4 // 4
        self.N3 = self.W4
        # L4: Sy=2
        self.Ho, self.Wo = self.H4 - 2, self.W4 - 2  # output spatial
        assert self.Ho % 2 == 0
        self.NB4 = self.Ho // 2
        self.N4 = self.Wo


# ---------------------------------------------------------------------------
# Host-side prep: weight matrices and layer-1 row-im2col
# ---------------------------------------------------------------------------

def build_weights(w1, w2, w3, w4):
    s1, s2, s3, s4 = (np.sign(w).astype(np.float32) for w in (w1, w2, w3, w4))

    # L1: K = 54 (i*18+dy'), M = 128 (parity*64 + o*8 + t), r = 2t+parity
    l1 = np.zeros((54, 3, 128), np.float32)
    for dx in range(3):
        for i in range(3):
            for dyp in range(18):
                for parity in range(2):
                    for o in range(8):
                        for t in range(8):
                            r = 2 * t + parity
                            dy = dyp - r
                            if 0 <= dy <= 2:
                                l1[i * 18 + dyp, dx, parity * 64 + t * 8 + o] = \
                                    s1[o, i, dy, dx]

    def mk(s, Cin, Cout, Rep, Sy, Mt):
        # K = Cin*Rep (dy'*Cin+i), M = Cout*Sy (t*Cout+o)
        m = np.zeros((Cin * Rep, 3, Mt), np.float32)
        for dx in range(3):
            for i in range(Cin):
                for dyp in range(Rep):
                    for o in range(Cout):
                        for t in range(Sy):
                            dy = dyp - t
                            if 0 <= dy <= 2:
                                m[dyp * Cin + i, dx, t * Cout + o] = s[o, i, dy, dx]
        return m

    l2 = mk(s2, 8, 16, 10, 8, 128)
    l3 = mk(s3, 16, 32, 6, 4, 128)
    l4 = mk(s4, 32, 2, 4, 2, 4)
    return (l1.astype(np.float32), l2.astype(NP_BF16),
            l3.astype(NP_FP8), l4.astype(NP_FP8))


def build_r1(xb, cfg):
    # xb: [B,3,H,W] fp32 -> padded with 1.0 -> R1 [54, B, NB1, Wp]
    B = xb.shape[0]
    xpad = np.pad(xb, ((0, 0), (0, 0), (1, 1), (1, 1)), constant_values=1.0)
    r1 = np.empty((54, B, cfg.NB1, cfg.Wp), np.float32)
    for i in range(3):
        for dyp in range(18):
            # rows 16*blk + dyp for blk in 0..NB1-1
            rows = 16 * np.arange(cfg.NB1) + dyp
            r1[i * 18 + dyp] = xpad[:, i, rows, :]
    return r1


# ---------------------------------------------------------------------------
# Device kernel builder
# ---------------------------------------------------------------------------

def build_bcnn(ctx, tc, outs, ins, cfg):
    nc = tc.nc
    B = cfg.B
    r1_d = ins["r1"]
    out_d = outs["out"]

    wpool = ctx.enter_context(tc.tile_pool(name="weights", bufs=1))
    w1 = wpool.tile([54, 3, 128], F32)
    nc.sync.dma_start(w1[:], ins["w1"])
    w2 = wpool.tile([80, 3, 128], BF16)
    nc.sync.dma_start(w2[:], ins["w2"])
    w3 = wpool.tile([96, 3, 128], FP8)
    nc.sync.dma_start(w3[:], ins["w3"])
    w4 = wpool.tile([128, 3, 4], FP8)
    nc.sync.dma_start(w4[:], ins["w4"])

    psum = ctx.enter_context(tc.tile_pool(name="psum", bufs=2, space="PSUM"))
    stage = ctx.enter_context(tc.tile_pool(name="stage", bufs=4))

    # ---------------- Layer 1 + pool + sign -> a2 ----------------
    a2_ctx = ExitStack()
    a2_pool = a2_ctx.enter_context(tc.tile_pool(name="a2", bufs=1, side="left"))
    a2 = a2_pool.tile([64, B, cfg.NB1, cfg.W2], BF16)

    CH1 = 4  # R1 blocks per DMA chunk
    with tc.tile_pool(name="r1c", bufs=3, side="left") as r1pool:
        for b in range(B):
            for blk0 in range(0, cfg.NB1, CH1):
                nch = min(CH1, cfg.NB1 - blk0)
                r1c = r1pool.tile([54, CH1, cfg.Wp], F32, tag="r1c")
                nc.sync.dma_start(r1c[:, :nch, :],
                                  r1_d[:, b, blk0:blk0 + nch, :])
                for j in range(nch):
                    ps = psum.tile([128, cfg.N1], F32, tag="ps1")
                    for dx in range(3):
                        nc.tensor.matmul(ps[:],
                                         w1[:, dx, :],
                                         r1c[:, j, dx:dx + cfg.N1],
                                         start=(dx == 0), stop=(dx == 2))
                    s1 = stage.tile([128, cfg.N1], BF16, tag="s1")
                    nc.scalar.activation(s1[:], ps[:],
                                         mybir.ActivationFunctionType.Sign)
                    sx = stage.tile([128, cfg.N1 // 2], BF16, tag="sx")
                    nc.vector.tensor_max(sx[:], s1[:, 0::2], s1[:, 1::2])
                    # 2-input engine ops need equal base partitions: copy the
                    # upper half (pool parity 1) down to partition 0 first.
                    sh = stage.tile([64, cfg.N1 // 2], BF16, tag="sh")
                    nc.vector.tensor_copy(sh[:], sx[64:128, :])
                    nc.vector.tensor_max(a2[:, b, blk0 + j, :],
                                         sx[0:64, :], sh[:])

    # ---------------- R2 build (SBUF->SBUF DMA) ----------------
    r2_ctx = ExitStack()
    r2_pool = r2_ctx.enter_context(tc.tile_pool(name="r2", bufs=1, side="right"))
    r2 = r2_pool.tile([80, B, cfg.NB2, cfg.W2], BF16)
    for dyp in range(8):
        # rows 8*b2+dyp live at a2 partitions [dyp*8, dyp*8+8), block b2
        nc.sync.dma_start(r2[dyp * 8:dyp * 8 + 8, :, :, :],
                          a2[dyp * 8:dyp * 8 + 8, :, 0:cfg.NB2, :])
    for dyp in (8, 9):
        e = dyp - 8
        nc.sync.dma_start(r2[dyp * 8:dyp * 8 + 8, :, 0:cfg.NB2 - 1, :],
                          a2[e * 8:e * 8 + 8, :, 1:cfg.NB2, :])
    nc.vector.memset(r2[64:80, :, cfg.NB2 - 1, :], 0.0)
    a2_ctx.close()

    # ---------------- Layer 2 -> a3 = sign(c2) ----------------
    a3_ctx = ExitStack()
    a3_pool = a3_ctx.enter_context(tc.tile_pool(name="a3", bufs=1, side="left"))
    a3 = a3_pool.tile([128, B, cfg.NB2, cfg.N2], FP8)
    for b in range(B):
        for b2 in range(0, cfg.NB2, 2):
            nbb = min(2, cfg.NB2 - b2)
            ps = psum.tile([128, 2, cfg.N2], F32, tag="ps2")
            for dx in range(3):
                nc.tensor.matmul(ps[:, :nbb, :],
                                 w2[:, dx, :],
                                 r2[:, b, b2:b2 + nbb, dx:dx + cfg.N2],
                                 start=(dx == 0), stop=(dx == 2))
            nc.scalar.activation(a3[:, b, b2:b2 + nbb, :], ps[:, :nbb, :],
                                 mybir.ActivationFunctionType.Sign)
    r2_ctx.close()

    # ---------------- R3 build ----------------
    r3_ctx = ExitStack()
    r3_pool = r3_ctx.enter_context(tc.tile_pool(name="r3", bufs=1, side="right"))
    r3 = r3_pool.tile([96, B, cfg.NB3, cfg.W3], FP8)
    n_even = (cfg.NB3 + 1) // 2
    n_odd = cfg.NB3 // 2
    for dyp in range(6):
        for b in range(B):
            # even b3=2m: row 8m+dyp -> a3 partitions [dyp*16, +16), block m
            nc.sync.dma_start(r3[dyp * 16:dyp * 16 + 16, b, 0:cfg.NB3:2, :],
                              a3[dyp * 16:dyp * 16 + 16, b, 0:n_even, 0:cfg.W3])
            # odd b3=2m+1: row 8m+4+dyp
            if dyp <= 3:
                nc.sync.dma_start(
                    r3[dyp * 16:dyp * 16 + 16, b, 1:cfg.NB3:2, :],
                    a3[(dyp + 4) * 16:(dyp + 5) * 16, b, 0:n_odd, 0:cfg.W3])
            else:
                nc.sync.dma_start(
                    r3[dyp * 16:dyp * 16 + 16, b, 1:cfg.NB3:2, :],
                    a3[(dyp - 4) * 16:(dyp - 3) * 16, b, 1:1 + n_odd, 0:cfg.W3])
    a3_ctx.close()

    # ---------------- Layer 3 -> a4 = sign(c3) ----------------
    a4_ctx = ExitStack()
    a4_pool = a4_ctx.enter_context(tc.tile_pool(name="a4", bufs=1, side="left"))
    a4 = a4_pool.tile([128, B, cfg.NB3, cfg.N3], FP8)
    for b in range(B):
        for b3 in range(0, cfg.NB3, 2):
            nbb = min(2, cfg.NB3 - b3)
            ps = psum.tile([128, 2, cfg.N3], F32, tag="ps3")
            for dx in range(3):
                nc.tensor.matmul(ps[:, :nbb, :],
                                 w3[:, dx, :],
                                 r3[:, b, b3:b3 + nbb, dx:dx + cfg.N3],
                                 start=(dx == 0), stop=(dx == 2))
            nc.scalar.activation(a4[:, b, b3:b3 + nbb, :], ps[:, :nbb, :],
                                 mybir.ActivationFunctionType.Sign)
    r3_ctx.close()

    # ---------------- R4 build ----------------
    r4_ctx = ExitStack()
    r4_pool = r4_ctx.enter_context(tc.tile_pool(name="r4", bufs=1, side="right"))
    r4 = r4_pool.tile([128, B, cfg.NB4, cfg.W4], FP8)
    n_even = (cfg.NB4 + 1) // 2
    n_odd = cfg.NB4 // 2
    for dyp in range(4):
        for b in range(B):
            # even b4=2m: row 4m+dyp -> a4 partitions [dyp*32, +32), block m
            nc.sync.dma_start(r4[dyp * 32:dyp * 32 + 32, b, 0:cfg.NB4:2, :],
                              a4[dyp * 32:dyp * 32 + 32, b, 0:n_even, 0:cfg.W4])
            # odd b4=2m+1: row 4m+2+dyp
            if dyp <= 1:
                nc.sync.dma_start(
                    r4[dyp * 32:dyp * 32 + 32, b, 1:cfg.NB4:2, :],
                    a4[(dyp + 2) * 32:(dyp + 3) * 32, b, 0:n_odd, 0:cfg.W4])
            else:
                nc.sync.dma_start(
                    r4[dyp * 32:dyp * 32 + 32, b, 1:cfg.NB4:2, :],
                    a4[(dyp - 2) * 32:(dyp - 1) * 32, b, 1:1 + n_odd, 0:cfg.W4])
    a4_ctx.close()

    # ---------------- Layer 4 -> out = clip(c4, -1, 1) ----------------
    # out_d: [B, 2, Ho, Wo]; psum partitions p = t*2+o
    for b in range(B):
        for b4 in range(0, cfg.NB4, 2):
            nbb = min(2, cfg.NB4 - b4)
            ps = psum.tile([4, 2, cfg.N4], F32, tag="ps4")
            for dx in range(3):
                nc.tensor.matmul(ps[:, :nbb, :],
                                 w4[:, dx, :],
                                 r4[:, b, b4:b4 + nbb, dx:dx + cfg.N4],
                                 start=(dx == 0), stop=(dx == 2))
            ob = stage.tile([4, 2, cfg.N4], F32, tag="ob")
            nc.vector.tensor_scalar(ob[:, :nbb, :], ps[:, :nbb, :],
                                    -1.0, 1.0,
                                    mybir.AluOpType.max, mybir.AluOpType.min)
            # sbuf partitions (t,o) x free (bb, x) -> out[b, o, 2*(b4+bb)+t, x]
            for t in range(2):
                nc.sync.dma_start(
                    out_d[b, :, 2 * b4 + t:2 * b4 + t + 2 * nbb - 1:2, :],
                    ob[t * 2:t * 2 + 2, :nbb, :])
    r4_ctx.close()


# ---------------------------------------------------------------------------
# Program build + run
# ---------------------------------------------------------------------------

_CACHE = {}


def _get_program(cfg):
    key = (cfg.B, cfg.H, cfg.W)
    if key in _CACHE:
        return _CACHE[key]
    nc = bacc.Bacc("TRN2", target_bir_lowering=False, debug=False,
                   num_devices=8)
    ins = {
        "r1": nc.dram_tensor("r1", [54, cfg.B, cfg.NB1, cfg.Wp], F32,
                             kind="ExternalInput").ap(),
        "w1": nc.dram_tensor("w1", [54, 3, 128], F32,
                             kind="ExternalInput").ap(),
        "w2": nc.dram_tensor("w2", [80, 3, 128], BF16,
                             kind="ExternalInput").ap(),
        "w3": nc.dram_tensor("w3", [96, 3, 128], FP8,
                             kind="ExternalInput").ap(),
        "w4": nc.dram_tensor("w4", [128, 3, 4], FP8,
                             kind="ExternalInput").ap(),
    }
    outs = {
        "out": nc.dram_tensor("out", [cfg.B, 2, cfg.Ho, cfg.Wo], F32,
                              kind="ExternalOutput").ap(),
    }
    with tile.TileContext(nc) as tc:
        with ExitStack() as ctx:
            build_bcnn(ctx, tc, outs, ins, cfg)
    nc.compile()
    _CACHE[key] = nc
    return nc


def kernel(x, w1, w2, w3, w4, _trace=False, _tmpdir=None):
    x = np.asarray(x, np.float32)
    NCORES = 8
    B_total = x.shape[0]
    Bc = B_total // NCORES
    cfg = Cfg(Bc, x.shape[2], x.shape[3])
    nc = _get_program(cfg)

    l1, l2, l3, l4 = build_weights(np.asarray(w1), np.asarray(w2),
                                   np.asarray(w3), np.asarray(w4))
    in_maps = []
    for c in range(NCORES):
        xb = x[c * Bc:(c + 1) * Bc]
        in_maps.append({
            "r1": build_r1(xb, cfg),
            "w1": l1, "w2": l2, "w3": l3, "w4": l4,
        })

    from concourse.bass_utils import run_bass_kernel_spmd
    res = run_bass_kernel_spmd(nc, in_maps, core_ids=list(range(NCORES)),
                               trace=_trace, tmpdir=_tmpdir)
    out = np.concatenate([res.results[c]["out"] for c in range(NCORES)], axis=0)
    out = out.reshape(B_total, -1).astype(np.float32)
    kernel._last_exec_time_ns = res.exec_time_ns
    return out


# revision 14
# speedup vs baseline: 1.7613x; 1.7613x over previous
# Binarized CNN (MCNET) on 8 TRN2 NeuronCores — pure batch data-parallel.
#
# Math: reference net is
#   h = pad(x, 1, value=1)
#   c1 = conv3x3(h, sign(w1)); a2 = maxpool2(hardtanh(c1)); (sign taken later)
#   c2 = conv3x3(sign(a2), sign(w2)); ...
#   out = hardtanh(c4)
# hardtanh is monotone and sign(hardtanh(v)) == sign(v), and
# sign(maxpool(v)) == maxpool(sign(v)), so the net collapses to:
#   c1 (fp32) -> a2 = maxpool2(sign(c1)) in {-1,0,1}
#   c2 = conv(a2, sign(w2)) (exact small-integer arithmetic) -> a3 = sign(c2)
#   c3 = conv(a3, sign(w3)) -> a4 = sign(c3)
#   out = clip(conv(a4, sign(w4)), -1, 1)
# Layers 2-4 are exact in any fp format with fp32 accumulation (values are
# ternary, sums <= 288), so they run in bf16/fp8. Layer 1 runs in fp32.
#
# Each conv layer is a matmul with partitions K = (channel, replicated-row),
# free dim = (row-block, x). dy is baked into the K replicas ("row im2col"),
# dx becomes 3 PSUM-accumulated matmuls with shifted rhs. Layer 1 packs the
# 2x2 maxpool parity into the PSUM partition order so pooling is a
# contiguous-partition-range max plus a strided free-dim max.

import numpy as np
import ml_dtypes
from contextlib import ExitStack

import concourse.bass as bass
import concourse.mybir as mybir
import concourse.tile as tile
from concourse import bacc

F32 = mybir.dt.float32
BF16 = mybir.dt.bfloat16
FP8 = mybir.dt.float8e4

NP_BF16 = ml_dtypes.bfloat16
NP_FP8 = ml_dtypes.float8_e4m3


class Cfg:
    def __init__(self, B, H, W):
        assert H % 16 == 0 and W % 16 == 0
        self.B, self.H, self.W = B, H, W
        self.Hp, self.Wp = H + 2, W + 2
        # L1: 16 conv rows per block -> 8 pooled rows per block
        self.NB1 = H // 16
        self.H2, self.W2 = H // 2, W // 2          # a2 spatial
        self.N1 = W                                 # L1 matmul free dim
        # L2: Sy=8
        self.H3, self.W3 = self.H2 - 2, self.W2 - 2  # c2/a3 spatial
        self.NB2 = (self.H3 + 7) // 8
        self.N2 = self.W3
        # L3: Sy=4
        self.H4, self.W4 = self.H3 - 2, self.W3 - 2  # c3/a4 spatial
        assert self.H4 % 4 == 0
        self.NB3 = self.H4 // 4
        self.N3 = self.W4
        # L4: Sy=2
        self.Ho, self.Wo = self.H4 - 2, self.W4 - 2  # output spatial
        assert self.Ho % 2 == 0
        self.NB4 = self.Ho // 2
        self.N4 = self.Wo


# ---------------------------------------------------------------------------
# Host-side prep: weight matrices and layer-1 row-im2col
# ---------------------------------------------------------------------------

def build_weights(w1, w2, w3, w4):
    s1, s2, s3, s4 = (np.sign(w).astype(np.float32) for w in (w1, w2, w3, w4))

    # L1: K = 162 (part*54 + i*18 + dy'), part in {hi,lo,lo2} of the bf16
    # split of x; M = 128 (parity*64 + t*8 + o), conv row r = 2t+parity
    l1 = np.zeros((162, 3, 128), np.float32)
    for dx in range(3):
        for i in range(3):
            for dyp in range(18):
                for parity in range(2):
                    for o in range(8):
                        for t in range(8):
                            r = 2 * t + parity
                            dy = dyp - r
                            if 0 <= dy <= 2:
                                for part in range(3):
                                    l1[part * 54 + i * 18 + dyp, dx,
                                       parity * 64 + t * 8 + o] = s1[o, i, dy, dx]

    def mk(s, Cin, Cout, Rep, Sy, Mt):
        # K = Cin*Rep (dy'*Cin+i), M = Cout*Sy (t*Cout+o)
        m = np.zeros((Cin * Rep, 3, Mt), np.float32)
        for dx in range(3):
            for i in range(Cin):
                for dyp in range(Rep):
                    for o in range(Cout):
                        for t in range(Sy):
                            dy = dyp - t
                            if 0 <= dy <= 2:
                                m[dyp * Cin + i, dx, t * Cout + o] = s[o, i, dy, dx]
        return m

    l2 = mk(s2, 8, 16, 10, 8, 128)
    l3 = mk(s3, 16, 32, 6, 4, 128)
    l4 = mk(s4, 32, 2, 4, 2, 4)
    return (l1.astype(NP_BF16), l2.astype(NP_BF16),
            l3.astype(NP_FP8), l4.astype(NP_FP8))


def build_r1(xb, cfg):
    # xb: [B,3,H,W] fp32 -> padded with 1.0, split exactly into three bf16
    # parts (x == hi + lo + lo2 in fp32) -> R1 [162, B, NB1, Wp] bf16
    B = xb.shape[0]
    xpad = np.pad(xb, ((0, 0), (0, 0), (1, 1), (1, 1)),
                  constant_values=1.0).astype(np.float32)
    hi = xpad.astype(NP_BF16)
    r = xpad - hi.astype(np.float32)
    lo = r.astype(NP_BF16)
    lo2 = (r - lo.astype(np.float32)).astype(NP_BF16)
    r1 = np.empty((162, B, cfg.NB1, cfg.Wp), NP_BF16)
    for part, arr in enumerate((hi, lo, lo2)):
        for i in range(3):
            for dyp in range(18):
                rows = 16 * np.arange(cfg.NB1) + dyp
                r1[part * 54 + i * 18 + dyp] = arr[:, i, rows, :]
    return r1


# ---------------------------------------------------------------------------
# Device kernel builder
# ---------------------------------------------------------------------------

def build_bcnn(ctx, tc, outs, ins, cfg):
    nc = tc.nc
    B = cfg.B
    r1_d = ins["r1"]
    out_d = outs["out"]
    Sign = mybir.ActivationFunctionType.Sign

    wpool = ctx.enter_context(tc.tile_pool(name="weights", bufs=1))
    w1a = wpool.tile([108, 3, 128], BF16)
    nc.sync.dma_start(w1a[:], ins["w1"][0:108])
    w1b = wpool.tile([54, 3, 128], BF16)
    nc.sync.dma_start(w1b[:], ins["w1"][108:162])
    w2 = wpool.tile([80, 3, 128], BF16)
    nc.sync.dma_start(w2[:], ins["w2"])
    w3 = wpool.tile([96, 3, 128], FP8)
    nc.sync.dma_start(w3[:], ins["w3"])
    w4 = wpool.tile([128, 3, 4], FP8)
    nc.sync.dma_start(w4[:], ins["w4"])

    stage = ctx.enter_context(tc.tile_pool(name="stage", bufs=4))

    # ---------------- Layer 1 + pool + sign -> a2 ----------------
    # x split into (hi, lo, lo2) bf16 parts: R1 rows 0:108 = hi+lo, 108:162
    # = lo2. Two matmuls per dx, weights held across the blocks of a chunk.
    a2_ctx = ExitStack()
    a2_pool = a2_ctx.enter_context(tc.tile_pool(name="a2", bufs=1, side="left"))
    a2 = a2_pool.tile([64, B, cfg.NB1, cfg.W2], BF16)

    CH1 = 4  # R1 blocks per chunk
    with tc.tile_pool(name="r1c", bufs=2, side="left") as r1pool, \
         tc.tile_pool(name="psum1", bufs=2 * CH1, space="PSUM") as psum1:
        for b in range(B):
            for blk0 in range(0, cfg.NB1, CH1):
                nch = min(CH1, cfg.NB1 - blk0)
                r1ca = r1pool.tile([108, CH1, cfg.Wp], BF16, tag="r1ca")
                nc.sync.dma_start(r1ca[:, :nch, :],
                                  r1_d[0:108, b, blk0:blk0 + nch, :])
                r1cb = r1pool.tile([54, CH1, cfg.Wp], BF16, tag="r1cb")
                nc.sync.dma_start(r1cb[:, :nch, :],
                                  r1_d[108:162, b, blk0:blk0 + nch, :])
                pss = [psum1.tile([128, cfg.N1], F32, tag="ps1", name="ps1")
                       for _ in range(nch)]
                for wt, rc, pg in ((w1a, r1ca, 0), (w1b, r1cb, 1)):
                    for dx in range(3):
                        for j in range(nch):
                            nc.tensor.matmul(pss[j][:],
                                             wt[:, dx, :],
                                             rc[:, j, dx:dx + cfg.N1],
                                             start=(pg == 0 and dx == 0),
                                             stop=(pg == 1 and dx == 2))
                for j in range(nch):
                    s1 = stage.tile([128, cfg.N1], BF16, tag="s1")
                    nc.scalar.activation(s1[:], pss[j][:], Sign)
                    sx = stage.tile([128, cfg.N1 // 2], BF16, tag="sx")
                    nc.vector.tensor_max(sx[:], s1[:, 0::2], s1[:, 1::2])
                    # 2-input engine ops need equal base partitions: copy the
                    # upper half (pool parity 1) down to partition 0 first.
                    sh = stage.tile([64, cfg.N1 // 2], BF16, tag="sh")
                    nc.vector.tensor_copy(sh[:], sx[64:128, :])
                    nc.vector.tensor_max(a2[:, b, blk0 + j, :],
                                         sx[0:64, :], sh[:])

    # ---------------- R2 build (SBUF->SBUF DMA) ----------------
    r2_ctx = ExitStack()
    r2_pool = r2_ctx.enter_context(tc.tile_pool(name="r2", bufs=1, side="right"))
    r2 = r2_pool.tile([80, B, cfg.NB2, cfg.W2], BF16)
    for dyp in range(8):
        # rows 8*b2+dyp live at a2 partitions [dyp*8, dyp*8+8), block b2
        nc.sync.dma_start(r2[dyp * 8:dyp * 8 + 8, :, :, :],
                          a2[dyp * 8:dyp * 8 + 8, :, 0:cfg.NB2, :])
    for dyp in (8, 9):
        e = dyp - 8
        nc.sync.dma_start(r2[dyp * 8:dyp * 8 + 8, :, 0:cfg.NB2 - 1, :],
                          a2[e * 8:e * 8 + 8, :, 1:cfg.NB2, :])
    nc.vector.memset(r2[64:80, :, cfg.NB2 - 1, :], 0.0)
    a2_ctx.close()

    # ---------------- Layer 2 -> a3 = sign(c2) ----------------
    a3_ctx = ExitStack()
    a3_pool = a3_ctx.enter_context(tc.tile_pool(name="a3", bufs=1, side="left"))
    a3 = a3_pool.tile([128, B, cfg.NB2, cfg.N2], FP8)
    iters2 = [(b, b2) for b in range(B) for b2 in range(0, cfg.NB2, 2)]
    G = 4
    with tc.tile_pool(name="psum2", bufs=2 * G, space="PSUM") as psum2:
        for g0 in range(0, len(iters2), G):
            grp = iters2[g0:g0 + G]
            pss = [psum2.tile([128, 2, cfg.N2], F32, tag="ps2", name="ps2") for _ in grp]
            for dx in range(3):
                for ps, (b, b2) in zip(pss, grp):
                    nbb = min(2, cfg.NB2 - b2)
                    nc.tensor.matmul(ps[:, :nbb, :],
                                     w2[:, dx, :],
                                     r2[:, b, b2:b2 + nbb, dx:dx + cfg.N2],
                                     start=(dx == 0), stop=(dx == 2))
            for ps, (b, b2) in zip(pss, grp):
                nbb = min(2, cfg.NB2 - b2)
                nc.scalar.activation(a3[:, b, b2:b2 + nbb, :],
                                     ps[:, :nbb, :], Sign)
    r2_ctx.close()

    # ---------------- R3 build ----------------
    r3_ctx = ExitStack()
    r3_pool = r3_ctx.enter_context(tc.tile_pool(name="r3", bufs=1, side="right"))
    r3 = r3_pool.tile([96, B, cfg.NB3, cfg.W3], FP8)
    n_even = (cfg.NB3 + 1) // 2
    n_odd = cfg.NB3 // 2
    for dyp in range(6):
        for b in range(B):
            # even b3=2m: row 8m+dyp -> a3 partitions [dyp*16, +16), block m
            nc.sync.dma_start(r3[dyp * 16:dyp * 16 + 16, b, 0:cfg.NB3:2, :],
                              a3[dyp * 16:dyp * 16 + 16, b, 0:n_even, 0:cfg.W3])
            # odd b3=2m+1: row 8m+4+dyp
            if dyp <= 3:
                nc.sync.dma_start(
                    r3[dyp * 16:dyp * 16 + 16, b, 1:cfg.NB3:2, :],
                    a3[(dyp + 4) * 16:(dyp + 5) * 16, b, 0:n_odd, 0:cfg.W3])
            else:
                nc.sync.dma_start(
                    r3[dyp * 16:dyp * 16 + 16, b, 1:cfg.NB3:2, :],
                    a3[(dyp - 4) * 16:(dyp - 3) * 16, b, 1:1 + n_odd, 0:cfg.W3])
    a3_ctx.close()

    # ---------------- Layer 3 -> a4 = sign(c3) ----------------
    a4_ctx = ExitStack()
    a4_pool = a4_ctx.enter_context(tc.tile_pool(name="a4", bufs=1, side="left"))
    a4 = a4_pool.tile([128, B, cfg.NB3, cfg.N3], FP8)
    iters3 = [(b, b3) for b in range(B) for b3 in range(0, cfg.NB3, 2)]
    with tc.tile_pool(name="psum3", bufs=2 * G, space="PSUM") as psum3:
        for g0 in range(0, len(iters3), G):
            grp = iters3[g0:g0 + G]
            pss = [psum3.tile([128, 2, cfg.N3], F32, tag="ps3", name="ps3") for _ in grp]
            for dx in range(3):
                for ps, (b, b3) in zip(pss, grp):
                    nbb = min(2, cfg.NB3 - b3)
                    nc.tensor.matmul(ps[:, :nbb, :],
                                     w3[:, dx, :],
                                     r3[:, b, b3:b3 + nbb, dx:dx + cfg.N3],
                                     start=(dx == 0), stop=(dx == 2))
            for ps, (b, b3) in zip(pss, grp):
                nbb = min(2, cfg.NB3 - b3)
                nc.scalar.activation(a4[:, b, b3:b3 + nbb, :],
                                     ps[:, :nbb, :], Sign)
    r3_ctx.close()

    # ---------------- R4 build (per image) + Layer 4 ----------------
    # r4 for one image is ~31.5KB/partition; build/compute per image with
    # double buffering instead of materializing all images at once.
    n_even = (cfg.NB4 + 1) // 2
    n_odd = cfg.NB4 // 2
    GB = 4
    r4_ctx = ExitStack()
    r4_pool = r4_ctx.enter_context(tc.tile_pool(name="r4p", bufs=2,
                                                side="right"))
    with tc.tile_pool(name="psum4", bufs=2 * GB, space="PSUM") as psum4:
        for b in range(B):
            r4 = r4_pool.tile([128, cfg.NB4, cfg.W4], FP8, tag="r4", name="r4")
            for dyp in range(4):
                # even b4=2m: row 4m+dyp -> a4 partitions [dyp*32, +32)
                nc.sync.dma_start(r4[dyp * 32:dyp * 32 + 32, 0:cfg.NB4:2, :],
                                  a4[dyp * 32:dyp * 32 + 32, b, 0:n_even,
                                     0:cfg.W4])
                # odd b4=2m+1: row 4m+2+dyp
                if dyp <= 1:
                    nc.sync.dma_start(
                        r4[dyp * 32:dyp * 32 + 32, 1:cfg.NB4:2, :],
                        a4[(dyp + 2) * 32:(dyp + 3) * 32, b, 0:n_odd, 0:cfg.W4])
                else:
                    nc.sync.dma_start(
                        r4[dyp * 32:dyp * 32 + 32, 1:cfg.NB4:2, :],
                        a4[(dyp - 2) * 32:(dyp - 1) * 32, b, 1:1 + n_odd,
                           0:cfg.W4])
            # psum partitions p = t*2+o; batch clip + output DMA per group
            iters4 = list(range(0, cfg.NB4, 2))
            for g0 in range(0, len(iters4), GB):
                grp = iters4[g0:g0 + GB]
                pss = [psum4.tile([4, 2, cfg.N4], F32, tag="ps4", name="ps4")
                       for _ in grp]
                for dx in range(3):
                    for ps, b4 in zip(pss, grp):
                        nbb = min(2, cfg.NB4 - b4)
                        nc.tensor.matmul(ps[:, :nbb, :],
                                         w4[:, dx, :],
                                         r4[:, b4:b4 + nbb, dx:dx + cfg.N4],
                                         start=(dx == 0), stop=(dx == 2))
                bs = grp[0]
                be = min(grp[-1] + 2, cfg.NB4)
                nb = be - bs
                ob = stage.tile([4, 2 * GB, cfg.N4], F32, tag="ob", bufs=2)
                off = 0
                for ps, b4 in zip(pss, grp):
                    nbb = min(2, cfg.NB4 - b4)
                    nc.vector.tensor_scalar(ob[:, off:off + nbb, :],
                                            ps[:, :nbb, :], -1.0, 1.0,
                                            mybir.AluOpType.max,
                                            mybir.AluOpType.min)
                    off += nbb
                for t in range(2):
                    nc.sync.dma_start(
                        out_d[b, :, 2 * bs + t:2 * bs + t + 2 * nb - 1:2, :],
                        ob[t * 2:t * 2 + 2, 0:nb, :])
    a4_ctx.close()
    r4_ctx.close()


# ---------------------------------------------------------------------------
# Program build + run
# ---------------------------------------------------------------------------

_CACHE = {}


def _get_program(cfg):
    key = (cfg.B, cfg.H, cfg.W)
    if key in _CACHE:
        return _CACHE[key]
    nc = bacc.Bacc("TRN2", target_bir_lowering=False, debug=False,
                   num_devices=8)
    ins = {
        "r1": nc.dram_tensor("r1", [162, cfg.B, cfg.NB1, cfg.Wp], BF16,
                             kind="ExternalInput").ap(),
        "w1": nc.dram_tensor("w1", [162, 3, 128], BF16,
                             kind="ExternalInput").ap(),
        "w2": nc.dram_tensor("w2", [80, 3, 128], BF16,
                             kind="ExternalInput").ap(),
        "w3": nc.dram_tensor("w3", [96, 3, 128], FP8,
                             kind="ExternalInput").ap(),
        "w4": nc.dram_tensor("w4", [128, 3, 4], FP8,
                             kind="ExternalInput").ap(),
    }
    outs = {
        "out": nc.dram_tensor("out", [cfg.B, 2, cfg.Ho, cfg.Wo], F32,
                              kind="ExternalOutput").ap(),
    }
    with tile.TileContext(nc) as tc:
        with ExitStack() as ctx:
            build_bcnn(ctx, tc, outs, ins, cfg)
    nc.compile()
    _CACHE[key] = nc
    return nc


def kernel(x, w1, w2, w3, w4, _trace=False, _tmpdir=None):
    x = np.asarray(x, np.float32)
    NCORES = 8
    B_total = x.shape[0]
    Bc = B_total // NCORES
    cfg = Cfg(Bc, x.shape[2], x.shape[3])
    nc = _get_program(cfg)

    l1, l2, l3, l4 = build_weights(np.asarray(w1), np.asarray(w2),
                                   np.asarray(w3), np.asarray(w4))
    in_maps = []
    for c in range(NCORES):
        xb = x[c * Bc:(c + 1) * Bc]
        in_maps.append({
            "r1": build_r1(xb, cfg),
            "w1": l1, "w2": l2, "w3": l3, "w4": l4,
        })

    from concourse.bass_utils import run_bass_kernel_spmd
    res = run_bass_kernel_spmd(nc, in_maps, core_ids=list(range(NCORES)),
                               trace=_trace, tmpdir=_tmpdir)
    out = np.concatenate([res.results[c]["out"] for c in range(NCORES)], axis=0)
    out = out.reshape(B_total, -1).astype(np.float32)
    kernel._last_exec_time_ns = res.exec_time_ns
    return out


# revision 17
# speedup vs baseline: 2.2106x; 1.2551x over previous
# Binarized CNN (MCNET) on 8 TRN2 NeuronCores — pure batch data-parallel.
#
# Math: reference net is
#   h = pad(x, 1, value=1)
#   c1 = conv3x3(h, sign(w1)); a2 = maxpool2(hardtanh(c1)); (sign taken later)
#   c2 = conv3x3(sign(a2), sign(w2)); ...
#   out = hardtanh(c4)
# hardtanh is monotone and sign(hardtanh(v)) == sign(v), and
# sign(maxpool(v)) == maxpool(sign(v)), so the net collapses to:
#   c1 (fp32) -> a2 = maxpool2(sign(c1)) in {-1,0,1}
#   c2 = conv(a2, sign(w2)) (exact small-integer arithmetic) -> a3 = sign(c2)
#   c3 = conv(a3, sign(w3)) -> a4 = sign(c3)
#   out = clip(conv(a4, sign(w4)), -1, 1)
# Layers 2-4 are exact in any fp format with fp32 accumulation (values are
# ternary, sums <= 288), so they run in bf16/fp8. Layer 1 runs in fp32.
#
# Each conv layer is a matmul with partitions K = (channel, replicated-row),
# free dim = (row-block, x). dy is baked into the K replicas ("row im2col"),
# dx becomes 3 PSUM-accumulated matmuls with shifted rhs. Layer 1 packs the
# 2x2 maxpool parity into the PSUM partition order so pooling is a
# contiguous-partition-range max plus a strided free-dim max.

import numpy as np
import ml_dtypes
from contextlib import ExitStack

import concourse.bass as bass
import concourse.mybir as mybir
import concourse.tile as tile
from concourse import bacc

F32 = mybir.dt.float32
BF16 = mybir.dt.bfloat16
FP8 = mybir.dt.float8e4

NP_BF16 = ml_dtypes.bfloat16
NP_FP8 = ml_dtypes.float8_e4m3


class Cfg:
    def __init__(self, B, H, W):
        assert H % 16 == 0 and W % 16 == 0
        self.B, self.H, self.W = B, H, W
        self.Hp, self.Wp = H + 2, W + 2
        # L1: 16 conv rows per block -> 8 pooled rows per block
        self.NB1 = H // 16
        self.H2, self.W2 = H // 2, W // 2          # a2 spatial
        self.N1 = W                                 # L1 matmul free dim
        # L2: Sy=8
        self.H3, self.W3 = self.H2 - 2, self.W2 - 2  # c2/a3 spatial
        self.NB2 = (self.H3 + 7) // 8
        self.N2 = self.W3
        # L3: Sy=4
        self.H4, self.W4 = self.H3 - 2, self.W3 - 2  # c3/a4 spatial
        assert self.H4 % 4 == 0
        self.NB3 = self.H4 // 4
        self.N3 = self.W4
        # L4: Sy=2
        self.Ho, self.Wo = self.H4 - 2, self.W4 - 2  # output spatial
        assert self.Ho % 2 == 0
        self.NB4 = self.Ho // 2
        self.N4 = self.Wo


# ---------------------------------------------------------------------------
# Host-side prep: weight matrices and layer-1 row-im2col
# ---------------------------------------------------------------------------

def build_weights(w1, w2, w3, w4):
    s1, s2, s3, s4 = (np.sign(w).astype(np.float32) for w in (w1, w2, w3, w4))

    # L1: K = 162 (part*54 + i*18 + dy'), part in {hi,lo,lo2} of the bf16
    # split of x; M = 128 (parity*64 + t*8 + o), conv row r = 2t+parity
    l1 = np.zeros((162, 3, 128), np.float32)
    for dx in range(3):
        for i in range(3):
            for dyp in range(18):
                for parity in range(2):
                    for o in range(8):
                        for t in range(8):
                            r = 2 * t + parity
                            dy = dyp - r
                            if 0 <= dy <= 2:
                                for part in range(3):
                                    l1[part * 54 + i * 18 + dyp, dx,
                                       parity * 64 + t * 8 + o] = s1[o, i, dy, dx]

    def mk(s, Cin, Cout, Rep, Sy, Mt):
        # K = Cin*Rep (dy'*Cin+i), M = Cout*Sy (t*Cout+o)
        m = np.zeros((Cin * Rep, 3, Mt), np.float32)
        for dx in range(3):
            for i in range(Cin):
                for dyp in range(Rep):
                    for o in range(Cout):
                        for t in range(Sy):
                            dy = dyp - t
                            if 0 <= dy <= 2:
                                m[dyp * Cin + i, dx, t * Cout + o] = s[o, i, dy, dx]
        return m

    l2 = mk(s2, 8, 16, 10, 8, 128)
    l3 = mk(s3, 16, 32, 6, 4, 128)
    l4 = mk(s4, 32, 2, 4, 2, 4)
    return (l1.astype(NP_BF16), l2.astype(NP_BF16),
            l3.astype(NP_FP8), l4.astype(NP_FP8))


def build_r1(xb, cfg):
    # xb: [B,3,H,W] fp32 -> padded with 1.0, split exactly into three bf16
    # parts (x == hi + lo + lo2 in fp32) -> R1 [162, B, NB1, Wp] bf16
    B = xb.shape[0]
    xpad = np.pad(xb, ((0, 0), (0, 0), (1, 1), (1, 1)),
                  constant_values=1.0).astype(np.float32)
    hi = xpad.astype(NP_BF16)
    r = xpad - hi.astype(np.float32)
    lo = r.astype(NP_BF16)
    lo2 = (r - lo.astype(np.float32)).astype(NP_BF16)
    r1 = np.empty((162, B, cfg.NB1, cfg.Wp), NP_BF16)
    for part, arr in enumerate((hi, lo, lo2)):
        for i in range(3):
            for dyp in range(18):
                rows = 16 * np.arange(cfg.NB1) + dyp
                r1[part * 54 + i * 18 + dyp] = arr[:, i, rows, :]
    return r1


# ---------------------------------------------------------------------------
# Device kernel builder
# ---------------------------------------------------------------------------

def build_bcnn(ctx, tc, outs, ins, cfg):
    nc = tc.nc
    B = cfg.B
    r1_d = ins["r1"]
    out_d = outs["out"]
    Sign = mybir.ActivationFunctionType.Sign
    G = 4

    wpool = ctx.enter_context(tc.tile_pool(name="weights", bufs=1))
    w1a = wpool.tile([108, 3, 128], BF16)
    nc.sync.dma_start(w1a[:], ins["w1"][0:108])
    w1b = wpool.tile([54, 3, 128], BF16)
    nc.sync.dma_start(w1b[:], ins["w1"][108:162])
    w2 = wpool.tile([80, 3, 128], BF16)
    nc.sync.dma_start(w2[:], ins["w2"])
    w3 = wpool.tile([96, 3, 128], FP8)
    nc.sync.dma_start(w3[:], ins["w3"])
    w4 = wpool.tile([128, 3, 4], FP8)
    nc.sync.dma_start(w4[:], ins["w4"])

    stage = ctx.enter_context(tc.tile_pool(name="stage", bufs=3))

    # Phase pipeline: each image's im2col rebuild (R2/R3/R4, SBUF->SBUF on
    # the gpsimd SWDGE queue) is emitted right after that image's compute,
    # so it overlaps the next image's matmul stream and the PE never waits.

    # ---------------- Phase 1: L1 + pool + sign -> a2, then R2(b) ----------
    a2_ctx = ExitStack()
    a2_pool = a2_ctx.enter_context(tc.tile_pool(name="a2", bufs=1, side="left"))
    a2 = a2_pool.tile([64, B, cfg.NB1, cfg.W2], BF16)
    r2_ctx = ExitStack()
    r2p = r2_ctx.enter_context(tc.tile_pool(name="r2p", bufs=4, side="right"))
    r2_tiles = {}

    CH1 = 4  # R1 blocks per chunk
    with tc.tile_pool(name="r1c", bufs=2, side="left") as r1pool, \
         tc.tile_pool(name="psum1", bufs=2 * CH1, space="PSUM") as psum1:
        for b in range(B):
            for blk0 in range(0, cfg.NB1, CH1):
                nch = min(CH1, cfg.NB1 - blk0)
                r1ca = r1pool.tile([108, CH1, cfg.Wp], BF16, tag="r1ca")
                nc.sync.dma_start(r1ca[:, :nch, :],
                                  r1_d[0:108, b, blk0:blk0 + nch, :])
                r1cb = r1pool.tile([54, CH1, cfg.Wp], BF16, tag="r1cb")
                nc.sync.dma_start(r1cb[:, :nch, :],
                                  r1_d[108:162, b, blk0:blk0 + nch, :])
                pss = [psum1.tile([128, cfg.N1], F32, tag="ps1", name="ps1")
                       for _ in range(nch)]
                for wt, rc, pg in ((w1a, r1ca, 0), (w1b, r1cb, 1)):
                    for dx in range(3):
                        for j in range(nch):
                            nc.tensor.matmul(pss[j][:],
                                             wt[:, dx, :],
                                             rc[:, j, dx:dx + cfg.N1],
                                             start=(pg == 0 and dx == 0),
                                             stop=(pg == 1 and dx == 2))
                for j in range(nch):
                    s1 = stage.tile([128, cfg.N1], BF16, tag="s1")
                    nc.scalar.activation(s1[:], pss[j][:], Sign)
                    sx = stage.tile([128, cfg.N1 // 2], BF16, tag="sx")
                    nc.vector.tensor_max(sx[:], s1[:, 0::2], s1[:, 1::2])
                    # 2-input engine ops need equal base partitions: copy the
                    # upper half (pool parity 1) down to partition 0 first.
                    sh = stage.tile([64, cfg.N1 // 2], BF16, tag="sh")
                    nc.vector.tensor_copy(sh[:], sx[64:128, :])
                    nc.vector.tensor_max(a2[:, b, blk0 + j, :],
                                         sx[0:64, :], sh[:])
            # R2 rebuild for this image (overlaps next image's L1)
            r2b = r2p.tile([80, cfg.NB2, cfg.W2], BF16, tag="r2", name="r2")
            for dyp in range(8):
                nc.gpsimd.dma_start(r2b[dyp * 8:dyp * 8 + 8, :, :],
                                    a2[dyp * 8:dyp * 8 + 8, b, 0:cfg.NB2, :])
            for dyp in (8, 9):
                e = dyp - 8
                nc.gpsimd.dma_start(r2b[dyp * 8:dyp * 8 + 8, 0:cfg.NB2 - 1, :],
                                    a2[e * 8:e * 8 + 8, b, 1:cfg.NB2, :])
            nc.vector.memset(r2b[64:80, cfg.NB2 - 1, :], 0.0)
            r2_tiles[b] = r2b
    a2_ctx.close()

    # ---------------- Phase 2: L2 -> a3 = sign(c2), then R3(b) -------------
    a3_ctx = ExitStack()
    a3_pool = a3_ctx.enter_context(tc.tile_pool(name="a3", bufs=1, side="left"))
    a3 = a3_pool.tile([128, B, cfg.NB2, cfg.N2], FP8)
    r3_ctx = ExitStack()
    r3p = r3_ctx.enter_context(tc.tile_pool(name="r3p", bufs=4, side="left"))
    r3_tiles = {}
    n_even3 = (cfg.NB3 + 1) // 2
    n_odd3 = cfg.NB3 // 2
    with tc.tile_pool(name="psum2", bufs=2 * G, space="PSUM") as psum2:
        for b in range(B):
            r2b = r2_tiles[b]
            iters2 = list(range(0, cfg.NB2, 2))
            for g0 in range(0, len(iters2), G):
                grp = iters2[g0:g0 + G]
                pss = [psum2.tile([128, 2, cfg.N2], F32, tag="ps2", name="ps2")
                       for _ in grp]
                for dx in range(3):
                    for ps, b2 in zip(pss, grp):
                        nbb = min(2, cfg.NB2 - b2)
                        nc.tensor.matmul(ps[:, :nbb, :],
                                         w2[:, dx, :],
                                         r2b[:, b2:b2 + nbb, dx:dx + cfg.N2],
                                         start=(dx == 0), stop=(dx == 2))
                for ps, b2 in zip(pss, grp):
                    nbb = min(2, cfg.NB2 - b2)
                    nc.scalar.activation(a3[:, b, b2:b2 + nbb, :],
                                         ps[:, :nbb, :], Sign)
            r3b = r3p.tile([96, cfg.NB3, cfg.W3], FP8, tag="r3", name="r3")
            for dyp in range(6):
                nc.gpsimd.dma_start(
                    r3b[dyp * 16:dyp * 16 + 16, 0:cfg.NB3:2, :],
                    a3[dyp * 16:dyp * 16 + 16, b, 0:n_even3, 0:cfg.W3])
                if dyp <= 3:
                    nc.gpsimd.dma_start(
                        r3b[dyp * 16:dyp * 16 + 16, 1:cfg.NB3:2, :],
                        a3[(dyp + 4) * 16:(dyp + 5) * 16, b, 0:n_odd3,
                           0:cfg.W3])
                else:
                    nc.gpsimd.dma_start(
                        r3b[dyp * 16:dyp * 16 + 16, 1:cfg.NB3:2, :],
                        a3[(dyp - 4) * 16:(dyp - 3) * 16, b, 1:1 + n_odd3,
                           0:cfg.W3])
            r3_tiles[b] = r3b
    r2_ctx.close()

    # ---------------- Phase 3: L3 -> a4 = sign(c3), then R4(b) -------------
    a4_ctx = ExitStack()
    a4p = a4_ctx.enter_context(tc.tile_pool(name="a4p", bufs=2, side="left"))
    r4_ctx = ExitStack()
    r4p = r4_ctx.enter_context(tc.tile_pool(name="r4p", bufs=2, side="right"))
    r4_tiles = {}
    n_even4 = (cfg.NB4 + 1) // 2
    n_odd4 = cfg.NB4 // 2
    with tc.tile_pool(name="psum3", bufs=2 * G, space="PSUM") as psum3:
        for b in range(B):
            r3b = r3_tiles[b]
            a4b = a4p.tile([128, cfg.NB3, cfg.N3], FP8, tag="a4", name="a4")
            iters3 = list(range(0, cfg.NB3, 2))
            for g0 in range(0, len(iters3), G):
                grp = iters3[g0:g0 + G]
                pss = [psum3.tile([128, 2, cfg.N3], F32, tag="ps3", name="ps3")
                       for _ in grp]
                for dx in range(3):
                    for ps, b3 in zip(pss, grp):
                        nbb = min(2, cfg.NB3 - b3)
                        nc.tensor.matmul(ps[:, :nbb, :],
                                         w3[:, dx, :],
                                         r3b[:, b3:b3 + nbb, dx:dx + cfg.N3],
                                         start=(dx == 0), stop=(dx == 2))
                for ps, b3 in zip(pss, grp):
                    nbb = min(2, cfg.NB3 - b3)
                    nc.scalar.activation(a4b[:, b3:b3 + nbb, :],
                                         ps[:, :nbb, :], Sign)
            r4b = r4p.tile([128, cfg.NB4, cfg.W4], FP8, tag="r4", name="r4")
            for dyp in range(4):
                nc.gpsimd.dma_start(
                    r4b[dyp * 32:dyp * 32 + 32, 0:cfg.NB4:2, :],
                    a4b[dyp * 32:dyp * 32 + 32, 0:n_even4, 0:cfg.W4])
                if dyp <= 1:
                    nc.gpsimd.dma_start(
                        r4b[dyp * 32:dyp * 32 + 32, 1:cfg.NB4:2, :],
                        a4b[(dyp + 2) * 32:(dyp + 3) * 32, 0:n_odd4, 0:cfg.W4])
                else:
                    nc.gpsimd.dma_start(
                        r4b[dyp * 32:dyp * 32 + 32, 1:cfg.NB4:2, :],
                        a4b[(dyp - 2) * 32:(dyp - 1) * 32, 1:1 + n_odd4,
                            0:cfg.W4])
            r4_tiles[b] = r4b
    a4_ctx.close()
    r3_ctx.close()
    a3_ctx.close()

    # ---------------- Phase 4: L4 -> out = clip(c4, -1, 1) -----------------
    # psum partitions p = t*2+o; batch clip + output DMA per group
    with tc.tile_pool(name="psum4", bufs=2 * G, space="PSUM") as psum4:
        for b in range(B):
            r4b = r4_tiles[b]
            iters4 = list(range(0, cfg.NB4, 2))
            for g0 in range(0, len(iters4), G):
                grp = iters4[g0:g0 + G]
                pss = [psum4.tile([4, 2, cfg.N4], F32, tag="ps4", name="ps4")
                       for _ in grp]
                for dx in range(3):
                    for ps, b4 in zip(pss, grp):
                        nbb = min(2, cfg.NB4 - b4)
                        nc.tensor.matmul(ps[:, :nbb, :],
                                         w4[:, dx, :],
                                         r4b[:, b4:b4 + nbb, dx:dx + cfg.N4],
                                         start=(dx == 0), stop=(dx == 2))
                bs = grp[0]
                be = min(grp[-1] + 2, cfg.NB4)
                nb = be - bs
                ob = stage.tile([4, 2 * G, cfg.N4], BF16, tag="ob", bufs=2)
                off = 0
                for ps, b4 in zip(pss, grp):
                    nbb = min(2, cfg.NB4 - b4)
                    nc.vector.tensor_scalar(ob[:, off:off + nbb, :],
                                            ps[:, :nbb, :], -1.0, 1.0,
                                            mybir.AluOpType.max,
                                            mybir.AluOpType.min)
                    off += nbb
                for t in range(2):
                    # SWDGE casts bf16 -> fp32 during the store (exact for
                    # the ternary output values)
                    nc.gpsimd.dma_start(
                        out_d[b, :, 2 * bs + t:2 * bs + t + 2 * nb - 1:2, :],
                        ob[t * 2:t * 2 + 2, 0:nb, :])
    r4_ctx.close()


# ---------------------------------------------------------------------------
# Program build + run
# ---------------------------------------------------------------------------

_CACHE = {}


def _get_program(cfg):
    key = (cfg.B, cfg.H, cfg.W)
    if key in _CACHE:
        return _CACHE[key]
    nc = bacc.Bacc("TRN2", target_bir_lowering=False, debug=False,
                   num_devices=8)
    ins = {
        "r1": nc.dram_tensor("r1", [162, cfg.B, cfg.NB1, cfg.Wp], BF16,
                             kind="ExternalInput").ap(),
        "w1": nc.dram_tensor("w1", [162, 3, 128], BF16,
                             kind="ExternalInput").ap(),
        "w2": nc.dram_tensor("w2", [80, 3, 128], BF16,
                             kind="ExternalInput").ap(),
        "w3": nc.dram_tensor("w3", [96, 3, 128], FP8,
                             kind="ExternalInput").ap(),
        "w4": nc.dram_tensor("w4", [128, 3, 4], FP8,
                             kind="ExternalInput").ap(),
    }
    outs = {
        "out": nc.dram_tensor("out", [cfg.B, 2, cfg.Ho, cfg.Wo], F32,
                              kind="ExternalOutput").ap(),
    }
    with tile.TileContext(nc) as tc:
        with ExitStack() as ctx:
            build_bcnn(ctx, tc, outs, ins, cfg)
    nc.compile()
    _CACHE[key] = nc
    return nc


def kernel(x, w1, w2, w3, w4, _trace=False, _tmpdir=None):
    x = np.asarray(x, np.float32)
    NCORES = 8
    B_total = x.shape[0]
    Bc = B_total // NCORES
    cfg = Cfg(Bc, x.shape[2], x.shape[3])
    nc = _get_program(cfg)

    l1, l2, l3, l4 = build_weights(np.asarray(w1), np.asarray(w2),
                                   np.asarray(w3), np.asarray(w4))
    in_maps = []
    for c in range(NCORES):
        xb = x[c * Bc:(c + 1) * Bc]
        in_maps.append({
            "r1": build_r1(xb, cfg),
            "w1": l1, "w2": l2, "w3": l3, "w4": l4,
        })

    from concourse.bass_utils import run_bass_kernel_spmd
    res = run_bass_kernel_spmd(nc, in_maps, core_ids=list(range(NCORES)),
                               trace=_trace, tmpdir=_tmpdir)
    out = np.concatenate([res.results[c]["out"] for c in range(NCORES)], axis=0)
    out = out.reshape(B_total, -1).astype(np.float32)
    kernel._last_exec_time_ns = res.exec_time_ns
    return out
